# revision 4
# baseline (speedup 1.0000x reference)
"""GCN (2-layer GraphConv x 2 graphs) on 8 Trainium2 NeuronCores — v2.

Sharding: 1D dst-node partition (6250 nodes/core). Each core processes the
edges whose dst lands in its slab. Layer 1 is computed as (A@X)@W0 (linearity
lets the spmm run on raw X), so the per-edge gather reads bf16 X rows via
gpsimd dma_gather. The segment-sum runs on the tensor engine: per 128-edge
tile a one-hot matrix M[e, slot] = vals[e]*(dst_local[e]==slot) is built with
one DVE tensor_scalar, and PSUM accumulates out^T[feat, slot] += msg^T @ M
per 128-node window.

v2 changes vs v1:
- Both graphs' h2 slabs packed into ONE [SLAB, 128] buffer (g1 cols 0:64,
  g2 cols 64:128) -> a single joint AllGather; layer 2 gathers straight from
  the AG output (no DRAM->DRAM re-pack copy).
- Single bf16 output tensor [128, SLAB] (g1 rows 0:64, g2 rows 64:128);
  output tensors cost ~230us each + ~22us/MB per exec in this runtime.
- Gather chunks for the two src halves are emitted interleaved so each PSUM
  window's tiles land early (PE starts sooner).
Edges are host-sorted by (dst window, src half); src halves keep dma_gather's
int16 indices in range. Per-(window,half) tile counts are padded to the max
across cores so all 8 cores share one SPMD program.
"""
import sys

sys.path.insert(0, "/opt/trn_rl_repo")

import numpy as np
import jax
import jax.numpy as jnp

N_NODES = 50000
N_EDGES = 600000
F_IN = 128
F_HID = 128
F_OUT = 64
C = 8
SLAB = N_NODES // C          # 6250
NWIN = (SLAB + 127) // 128   # 49
LAST_SLOTS = SLAB - 128 * (NWIN - 1)  # 106
HALF = N_NODES // 2          # 25000 (< 2^15 so int16 indices work)
CH_TILES = 7
MP_BUFS = 4
M_BUILD_TS = False           # fused per-tile build measured SLOWER (7x DVE instrs beat the saved pass)
SBUF_BUFS = 4
PS_BUFS = (3, 1, 3)          # psA (L1 spmm), psB (L1 flush, 2 tags), psC (L2 spmm); 8 banks; (3,1,3) beat (2,2,2) by ~0.5ms/exec
SINGLE_PACKET = True         # True caps CH_TILES at 7 (56+1 descs/engine <= 64/packet)
NQUEUES = 4                 # tiles/gather call: num_idxs/16+1 descs must fit the 64-desc DMA rings
DEBUG_GRAPHS = 2             # build only first N graphs
DEBUG_REPEAT = 1             # emit the whole pipeline K times (for timing)
DEBUG_SKIP_AG = False        # skip allgather (layer2 reads garbage)
DEBUG_SKIP_L2 = False        # skip layer 2 spmm entirely
DEBUG_L2_TABLE_GX = False    # timing-only: L2 gathers from gx instead of h2c
DEBUG_L1_LHS64 = False       # timing-only: L1 matmuls use 64-col lhsT

_bf16 = jnp.bfloat16


def _preprocess_graph(src, dst, vals):
    """Partition+sort edges; returns per-core streams and the shared plan."""
    src = np.asarray(src, np.int64)
    dst = np.asarray(dst, np.int64)
    vals = np.asarray(vals, np.float32)

    core = dst // SLAB
    dstl = dst % SLAB
    win = dstl // 128
    slot = dstl % 128
    half = (src >= HALF).astype(np.int64)
    idxr = (src - half * HALF).astype(np.int64)

    key = (core * 2 + half) * NWIN + win
    ngroups = C * 2 * NWIN
    counts = np.bincount(key, minlength=ngroups)
    tc = -(-counts // 128)  # ceil
    tmax = tc.reshape(C, 2, NWIN).max(axis=0)  # [2, NWIN]
    # every window needs at least one tile so its PSUM gets initialized
    tmax[0] = np.maximum(tmax[0], (tmax.sum(axis=0) == 0).astype(tmax.dtype))

    tile_off = np.zeros((2, NWIN), np.int64)
    tile_off[:, 1:] = np.cumsum(tmax, axis=1)[:, :-1]
    L = tmax.sum(axis=1) * 128  # edges per (core, half) stream, padded

    order = np.argsort(key, kind="stable")
    ksort = key[order]
    gstart = np.zeros(ngroups, np.int64)
    gstart[1:] = np.cumsum(counts)[:-1]
    cumcount = np.arange(len(src)) - gstart[ksort]

    csort = ksort // (2 * NWIN)
    hsort = (ksort // NWIN) % 2
    wsort = ksort % NWIN
    pos = tile_off[hsort, wsort] * 128 + cumcount

    streams = []
    for h in (0, 1):
        idx_a = np.zeros((C, L[h]), np.int16)
        sl_a = np.zeros((C, L[h]), np.float32)
        vl_a = np.zeros((C, L[h]), np.float32)
        m = hsort == h
        idx_a[csort[m], pos[m]] = idxr[order][m].astype(np.int16)
        sl_a[csort[m], pos[m]] = slot[order][m].astype(np.float32)
        vl_a[csort[m], pos[m]] = vals[order][m]
        streams.append((idx_a, sl_a, vl_a))

    plan = {
        "tmax": tmax,          # [2, NWIN] tile counts (shared across cores)
        "tile_off": tile_off,  # [2, NWIN] stream tile offsets
        "L": L,                # [2] padded stream lengths (edges)
    }
    return streams, plan


def _wrap_idx(a):
    # [L] int16 -> [128, L/16]: idx j at [j%16, j//16], replicated to 8 q7 cores
    L = a.shape[0]
    w = a.reshape(L // 16, 16).T
    return np.tile(w, (8, 1)).copy()


def _wrap128(a):
    # [L] -> [128, L/128]: edge j at [j%128, j//128]
    L = a.shape[0]
    return a.reshape(L // 128, 128).T.copy()


def _chunks(total_tiles):
    out = []
    p = 0
    while p < total_tiles:
        n = min(CH_TILES, total_tiles - p)
        out.append((p, n))
        p += n
    return out


class _GraphEmit:
    """Per-graph emission state: streams in SBUF, consts, plan."""

    def __init__(self, nc, pool, g, plan, tensors):
        from concourse import mybir

        self.nc = nc
        self.pool = pool
        self.g = g
        self.plan = plan
        (self.x_t, ixs, sls, vls, w0_t, w1_t, b0_t, b1_t, self.bb_t) = tensors
        sbuf, msgp, mp, idxp, slvp, consts, psA, psB, psC = pool

        tmax = plan["tmax"]
        self.stream_sb = []
        for h in (0, 1):
            total_tiles = int(tmax[h].sum())
            ix_s = idxp.tile([128, total_tiles * 8], mybir.dt.int16, tag=f"ixf{g}{h}")
            nc.sync.dma_start(out=ix_s[:], in_=ixs[h][:, :])
            sl_s = slvp.tile([128, total_tiles], mybir.dt.float32, tag=f"slf{g}{h}")
            nc.scalar.dma_start(out=sl_s[:], in_=sls[h][:, :])
            vl_s = slvp.tile([128, total_tiles], mybir.dt.float32, tag=f"vlf{g}{h}")
            nc.scalar.dma_start(out=vl_s[:], in_=vls[h][:, :])
            self.stream_sb.append((ix_s, sl_s, vl_s))

        self.w0_s = consts.tile([F_IN, F_HID], mybir.dt.bfloat16, tag=f"w0_{g}")
        nc.sync.dma_start(out=self.w0_s[:], in_=w0_t[:, :])
        self.w1_s = consts.tile([F_HID, F_OUT], mybir.dt.bfloat16, tag=f"w1_{g}")
        nc.sync.dma_start(out=self.w1_s[:], in_=w1_t[:, :])
        self.b0_s = consts.tile([F_HID, 1], mybir.dt.float32, tag=f"b0_{g}")
        nc.sync.dma_start(out=self.b0_s[:, 0:1], in_=b0_t[:, None])
        self.b1_s = consts.tile([F_OUT, 1], mybir.dt.float32, tag=f"b1_{g}")
        nc.sync.dma_start(out=self.b1_s[:, 0:1], in_=b1_t[:, None])
        # b1 replicated across partitions for the [slot, feat2] flush
        self.bb_s = consts.tile([128, F_OUT], mybir.dt.float32, tag=f"bb_{g}")
        nc.sync.dma_start(out=self.bb_s[:], in_=self.bb_t[:, :])


def _emit_spmm(nc, pool, ge, qctr, iota_s, table_t, feat, lcols, layer, flush):
    """Gather+M-build chunks (halves interleaved), then per-window matmul
    accumulation, calling flush(w, ps, slots) after each window's matmuls."""
    from concourse import mybir

    sbuf, msgp, mp, idxp, slvp, consts, psA, psB, psC = pool
    g = ge.g
    tmax, tile_off = ge.plan["tmax"], ge.plan["tile_off"]

    msg_chunks = [[], []]
    m_chunks = [[], []]
    chs = [_chunks(int(tmax[h].sum())) for h in (0, 1)]
    order = []
    for i in range(max(len(chs[0]), len(chs[1]))):
        for h in (0, 1):
            if i < len(chs[h]):
                order.append((h, chs[h][i]))
    for h, (p0, nt) in order:
        ix_s, sl_s, vl_s = ge.stream_sb[h]
        msg = msgp.tile([128, nt, feat], mybir.dt.bfloat16, tag=f"msg{h}")
        nc.gpsimd.dma_gather(
            out_ap=msg[:],
            in_ap=table_t[h * HALF:(h + 1) * HALF, :],
            idxs_ap=ix_s[:, p0 * 8:(p0 + nt) * 8],
            num_idxs=nt * 128,
            num_idxs_reg=nt * 128,
            elem_size=feat,
            queue_num=qctr[0] % NQUEUES,
            single_packet=SINGLE_PACKET,
        )
        qctr[0] += 1
        m_c = mp.tile([128, nt, 128], mybir.dt.bfloat16, tag=f"m{h}")
        if M_BUILD_TS:
            # one fused DVE op per tile: M = (iota == slot) * val, with slot
            # and val as per-partition scalars
            for t in range(nt):
                nc.vector.tensor_scalar(
                    out=m_c[:, t, :],
                    in0=iota_s[:, :],
                    scalar1=sl_s[:, p0 + t:p0 + t + 1],
                    scalar2=vl_s[:, p0 + t:p0 + t + 1],
                    op0=mybir.AluOpType.is_equal,
                    op1=mybir.AluOpType.mult,
                )
        else:
            nc.vector.tensor_tensor(
                out=m_c[:],
                in0=sl_s[:, p0:p0 + nt, None].to_broadcast([128, nt, 128]),
                in1=iota_s[:, None, :].to_broadcast([128, nt, 128]),
                op=mybir.AluOpType.is_equal,
            )
            nc.vector.tensor_tensor(
                out=m_c[:],
                in0=m_c[:],
                in1=vl_s[:, p0:p0 + nt, None].to_broadcast([128, nt, 128]),
                op=mybir.AluOpType.mult,
            )
        msg_chunks[h].append(msg)
        m_chunks[h].append(m_c)

    for w in range(NWIN):
        slots = 128 if w < NWIN - 1 else LAST_SLOTS
        tiles = []
        for h in (0, 1):
            for k in range(int(tmax[h][w])):
                p = int(tile_off[h][w]) + k
                tiles.append((h, p // CH_TILES, p % CH_TILES))
        if layer == 2:
            # flipped: stationary = M (128 cols -> FWL), moving = 64-col msg
            # slice; psum accumulates out[slot, feat2]
            ps = psC.tile([128, F_OUT], mybir.dt.float32,
                          space="PSUM", tag="ps_spmm2")
            for i, (h, q, t) in enumerate(tiles):
                nc.tensor.matmul(
                    out=ps[:slots, :],
                    lhsT=m_chunks[h][q][:, t, :slots],
                    rhs=msg_chunks[h][q][:, t, lcols],
                    start=(i == 0),
                    stop=(i == len(tiles) - 1),
                )
        else:
            ps = psA.tile([128, 128], mybir.dt.float32,
                          space="PSUM", tag="ps_spmm1")
            owidth = 128
            if lcols is not None:
                owidth = min(owidth, lcols.stop - lcols.start)
            for i, (h, q, t) in enumerate(tiles):
                msg = msg_chunks[h][q]
                lhsT = msg[:, t, :] if lcols is None else msg[:, t, lcols]
                nc.tensor.matmul(
                    out=ps[:owidth, :slots],
                    lhsT=lhsT,
                    rhs=m_chunks[h][q][:, t, :slots],
                    start=(i == 0),
                    stop=(i == len(tiles) - 1),
                )
        flush(w, ps, slots)


def _build_and_run(graphs):
    """graphs: list of (x, streams, plan, W0, b0, W1, b1) per graph."""
    from concourse import bacc, mybir, tile
    from concourse.bass_utils import run_bass_kernel_spmd

    nc = bacc.Bacc("TRN2", target_bir_lowering=False, debug=False,
                   num_devices=C, num_swdge_queues=NQUEUES)

    tensors_all = []
    for g, (x, streams, plan, W0, b0, W1, b1) in enumerate(graphs, start=1):
        x_t = nc.declare_dram_parameter(f"gx{g}", [N_NODES, F_IN], mybir.dt.bfloat16, isOutput=False)
        ixs, sls, vls = [], [], []
        for h in (0, 1):
            Lh = int(plan["L"][h])
            ixs.append(nc.declare_dram_parameter(f"ix{g}{h}", [128, Lh // 16], mybir.dt.int16, isOutput=False))
            sls.append(nc.declare_dram_parameter(f"sl{g}{h}", [128, Lh // 128], mybir.dt.float32, isOutput=False))
            vls.append(nc.declare_dram_parameter(f"vl{g}{h}", [128, Lh // 128], mybir.dt.float32, isOutput=False))
        w0_t = nc.declare_dram_parameter(f"w{g}0", [F_IN, F_HID], mybir.dt.bfloat16, isOutput=False)
        w1_t = nc.declare_dram_parameter(f"w{g}1", [F_HID, F_OUT], mybir.dt.bfloat16, isOutput=False)
        b0_t = nc.declare_dram_parameter(f"b{g}0", [F_HID], mybir.dt.float32, isOutput=False)
        b1_t = nc.declare_dram_parameter(f"b{g}1", [F_OUT], mybir.dt.float32, isOutput=False)
        bb_t = nc.declare_dram_parameter(f"bb{g}", [128, F_OUT], mybir.dt.float32, isOutput=False)
        tensors_all.append((x_t, ixs, sls, vls, w0_t, w1_t, b0_t, b1_t, bb_t))
    iota_t = nc.declare_dram_parameter("iota", [128, 128], mybir.dt.bfloat16, isOutput=False)
    # single packed output [node, 128]: graph g occupies cols (g-1)*64:g*64, bf16
    out_t = nc.declare_dram_parameter("out", [SLAB, 128], mybir.dt.bfloat16, isOutput=True)
    # joint h2 buffer: graph g occupies cols (g-1)*64:g*64; one AllGather for
    # both graphs (a second collective costs ~0.8ms/exec in this runtime)
    h2s_d = nc.dram_tensor("h2s", [SLAB, 128], mybir.dt.bfloat16)
    h2c_d = nc.dram_tensor("h2c", [N_NODES, 128], mybir.dt.bfloat16, addr_space="Shared")

    ngraphs = min(len(graphs), DEBUG_GRAPHS)

    with tile.TileContext(nc) as tc:
        with (
            tc.tile_pool(name="sbuf", bufs=SBUF_BUFS) as sbuf,
            tc.tile_pool(name="msgp", bufs=4) as msgp,
            tc.tile_pool(name="mp", bufs=MP_BUFS) as mp,
            tc.tile_pool(name="idxp", bufs=1) as idxp,
            tc.tile_pool(name="slvp", bufs=1) as slvp,
            tc.tile_pool(name="consts", bufs=1) as consts,
            tc.tile_pool(name="psA", bufs=PS_BUFS[0], space="PSUM") as psA,
            tc.tile_pool(name="psB", bufs=PS_BUFS[1], space="PSUM") as psB,
            tc.tile_pool(name="psC", bufs=PS_BUFS[2], space="PSUM") as psC,
        ):
            pool = (sbuf, msgp, mp, idxp, slvp, consts, psA, psB, psC)
            iota_s = consts.tile([128, 128], mybir.dt.bfloat16)
            nc.sync.dma_start(out=iota_s[:], in_=iota_t[:, :])
            qctr = [0]
            for _rep in range(DEBUG_REPEAT):
                ges = []
                for g in range(1, ngraphs + 1):
                    ge = _GraphEmit(nc, pool, g, graphs[g - 1][2], tensors_all[g - 1])
                    ges.append(ge)

                # ---- layer 1 for all graphs ----
                for ge in ges:
                    g = ge.g

                    def flush_l1(w, ps, slots, ge=ge, g=g):
                        axT = sbuf.tile([128, 128], mybir.dt.bfloat16, tag="axT")
                        nc.vector.tensor_copy(out=axT[:, :slots], in_=ps[:, :slots])
                        ps_mid = psB.tile([128, 128], mybir.dt.float32, space="PSUM", tag="ps_mid")
                        nc.tensor.matmul(out=ps_mid[:, :slots], lhsT=ge.w0_s[:],
                                         rhs=axT[:, :slots], start=True, stop=True)
                        r1t = sbuf.tile([128, 128], mybir.dt.bfloat16, tag="r1t")
                        nc.scalar.activation(out=r1t[:, :slots], in_=ps_mid[:, :slots],
                                             func=mybir.ActivationFunctionType.Relu,
                                             bias=ge.b0_s[:, 0:1])
                        ps_out = psB.tile([128, F_OUT], mybir.dt.float32, space="PSUM", tag="ps_out")
                        nc.tensor.matmul(out=ps_out[:slots, :], lhsT=r1t[:, :slots],
                                         rhs=ge.w1_s[:], start=True, stop=True)
                        h2sb = sbuf.tile([128, F_OUT], mybir.dt.bfloat16, tag="h2sb")
                        nc.vector.tensor_copy(out=h2sb[:slots, :], in_=ps_out[:slots, :])
                        eng = nc.sync if w % 2 == 0 else nc.scalar
                        eng.dma_start(
                            out=h2s_d[w * 128:w * 128 + slots, (g - 1) * F_OUT:g * F_OUT],
                            in_=h2sb[:slots, :])

                    l1cols = slice(0, F_OUT) if DEBUG_L1_LHS64 else None
                    _emit_spmm(nc, pool, ge, qctr, iota_s, ge.x_t, F_IN,
                               l1cols, 1, flush_l1)

                # ---- joint allgather ----
                if not DEBUG_SKIP_AG:
                    nc.gpsimd.collective_compute(
                        "AllGather",
                        mybir.AluOpType.bypass,
                        replica_groups=[list(range(C))],
                        ins=[h2s_d[:]],
                        outs=[h2c_d[:]],
                    )

                # ---- layer 2 for all graphs ----
                if not DEBUG_SKIP_L2:
                    for ge in ges:
                        g = ge.g

                        def flush_l2(w, ps, slots, g=g, ge=ge):
                            o_sb = sbuf.tile([128, F_OUT], mybir.dt.bfloat16, tag="o_sb")
                            nc.vector.tensor_tensor(
                                out=o_sb[:slots, :], in0=ps[:slots, :],
                                in1=ge.bb_s[:slots, :],
                                op=mybir.AluOpType.add)
                            eng = nc.sync if w % 2 == 0 else nc.scalar
                            eng.dma_start(
                                out=out_t[w * 128:w * 128 + slots,
                                          (g - 1) * F_OUT:g * F_OUT],
                                in_=o_sb[:slots, :])

                        table = h2c_d if not (DEBUG_SKIP_AG or DEBUG_L2_TABLE_GX) else ge.x_t
                        _emit_spmm(nc, pool, ge, qctr, iota_s, table, 128,
                                   slice((g - 1) * F_OUT, g * F_OUT), 2, flush_l2)

    nc.compile()

    # per-core input maps
    iota = np.tile(np.arange(128, dtype=np.float32), (128, 1))
    in_maps = []
    for c in range(C):
        m = {"iota": np.asarray(jnp.asarray(iota, _bf16))}
        for g, (x, streams, plan, W0, b0, W1, b1) in enumerate(graphs, start=1):
            m[f"gx{g}"] = np.asarray(jnp.asarray(x, _bf16))
            for h in (0, 1):
                idx_a, sl_a, vl_a = streams[h]
                m[f"ix{g}{h}"] = _wrap_idx(idx_a[c])
                m[f"sl{g}{h}"] = _wrap128(sl_a[c]).astype(np.float32)
                m[f"vl{g}{h}"] = _wrap128(vl_a[c]).astype(np.float32)
            m[f"w{g}0"] = np.asarray(jnp.asarray(W0, _bf16))
            m[f"w{g}1"] = np.asarray(jnp.asarray(W1, _bf16))
            m[f"b{g}0"] = np.asarray(b0, np.float32)
            m[f"b{g}1"] = np.asarray(b1, np.float32)
            m[f"bb{g}"] = np.tile(np.asarray(b1, np.float32)[None, :], (128, 1))
        in_maps.append(m)

    global _last_run
    _last_run = (nc, in_maps)
    res = run_bass_kernel_spmd(nc, in_maps, list(range(C)))
    return res.results


_last_run = None


def measure_exec_ns(n_iters=6):
    """Re-execute the last-built kernel with device-resident inputs; returns
    (t_min_ns, t_med_ns) over n_iters single-exec wall timings."""
    import time
    from jax.sharding import Mesh, PartitionSpec, NamedSharding
    from jax.experimental.shard_map import shard_map
    from concourse import mybir
    from concourse.bass2jax import _bass_exec_p, partition_id_tensor

    assert _last_run is not None
    nc, in_maps = _last_run
    partition_name = nc.partition_id_tensor.name if nc.partition_id_tensor else None

    in_names, out_names, out_avals, zero_shapes = [], [], [], []
    for alloc in nc.m.functions[0].allocations:
        if not isinstance(alloc, mybir.MemoryLocationSet):
            continue
        name = alloc.memorylocations[0].name
        if alloc.kind == "ExternalInput":
            if name != partition_name:
                in_names.append(name)
        elif alloc.kind == "ExternalOutput":
            out_names.append(name)
            shape = tuple(alloc.tensor_shape)
            dtype = mybir.dt.np(alloc.dtype)
            out_avals.append(jax.core.ShapedArray(shape, dtype))
            zero_shapes.append((shape, dtype))
    n_params = len(in_names)
    all_in_names = in_names + out_names
    if partition_name is not None:
        all_in_names = all_in_names + [partition_name]

    def _extra():
        return (partition_id_tensor(),) if partition_name is not None else ()

    def _body1(*args):
        return tuple(_bass_exec_p.bind(
            *args, *_extra(), out_avals=tuple(out_avals), in_names=tuple(all_in_names),
            out_names=tuple(out_names), lowering_input_output_aliases=(),
            sim_require_finite=True, sim_require_nnan=True, nc=nc))

    devices = jax.devices()[:C]
    mesh = Mesh(np.asarray(devices), ("core",))
    sh = NamedSharding(mesh, PartitionSpec("core"))

    concat_in = [np.concatenate([np.asarray(in_maps[c][nm]) for c in range(C)], axis=0)
                 for nm in in_names]
    dev_in = [jax.device_put(a, sh) for a in concat_in]

    specs = (PartitionSpec("core"),) * (n_params + len(out_avals))
    outs = (PartitionSpec("core"),) * len(out_avals)
    donate = tuple(range(n_params, n_params + len(out_avals)))
    f1 = jax.jit(shard_map(_body1, mesh=mesh, in_specs=specs, out_specs=outs,
                           check_rep=False),
                 donate_argnums=donate, keep_unused=True)

    def zeros():
        return [jax.device_put(np.zeros((C * s[0], *s[1:]), d), sh)
                for s, d in zero_shapes]

    o = f1(*dev_in, *zeros())
    jax.block_until_ready(o)
    t1 = []
    for _ in range(n_iters):
        z = zeros()
        jax.block_until_ready(z)
        t0 = time.perf_counter()
        o = f1(*dev_in, *z)
        jax.block_until_ready(o)
        t1.append(time.perf_counter() - t0)
    return min(t1) * 1e9, sorted(t1)[len(t1) // 2] * 1e9


def measure_pipelined_ns(n_iters=8):
    """Per-iter wall time of n_iters back-to-back execs (RTT amortized)."""
    import time
    from jax.sharding import Mesh, PartitionSpec, NamedSharding
    from jax.experimental.shard_map import shard_map
    from concourse import mybir
    from concourse.bass2jax import _bass_exec_p, partition_id_tensor

    assert _last_run is not None
    nc, in_maps = _last_run
    partition_name = nc.partition_id_tensor.name if nc.partition_id_tensor else None
    in_names, out_names, out_avals, zero_shapes = [], [], [], []
    for alloc in nc.m.functions[0].allocations:
        if not isinstance(alloc, mybir.MemoryLocationSet):
            continue
        name = alloc.memorylocations[0].name
        if alloc.kind == "ExternalInput":
            if name != partition_name:
                in_names.append(name)
        elif alloc.kind == "ExternalOutput":
            out_names.append(name)
            shape = tuple(alloc.tensor_shape)
            dtype = mybir.dt.np(alloc.dtype)
            out_avals.append(jax.core.ShapedArray(shape, dtype))
            zero_shapes.append((shape, dtype))
    n_params = len(in_names)
    all_in_names = in_names + out_names
    if partition_name is not None:
        all_in_names = all_in_names + [partition_name]

    def _extra():
        return (partition_id_tensor(),) if partition_name is not None else ()

    def _body1(*args):
        return tuple(_bass_exec_p.bind(
            *args, *_extra(), out_avals=tuple(out_avals), in_names=tuple(all_in_names),
            out_names=tuple(out_names), lowering_input_output_aliases=(),
            sim_require_finite=True, sim_require_nnan=True, nc=nc))

    devices = jax.devices()[:C]
    mesh = Mesh(np.asarray(devices), ("core",))
    sh = NamedSharding(mesh, PartitionSpec("core"))
    concat_in = [np.concatenate([np.asarray(in_maps[c][nm]) for c in range(C)], axis=0)
                 for nm in in_names]
    dev_in = [jax.device_put(a, sh) for a in concat_in]
    specs = (PartitionSpec("core"),) * (n_params + len(out_avals))
    outs = (PartitionSpec("core"),) * len(out_avals)
    donate = tuple(range(n_params, n_params + len(out_avals)))
    f1 = jax.jit(shard_map(_body1, mesh=mesh, in_specs=specs, out_specs=outs,
                           check_rep=False),
                 donate_argnums=donate, keep_unused=True)

    def zeros():
        return [jax.device_put(np.zeros((C * s[0], *s[1:]), d), sh)
                for s, d in zero_shapes]

    o = f1(*dev_in, *zeros())
    jax.block_until_ready(o)
    import time
    zs = [zeros() for _ in range(n_iters)]
    for z in zs:
        jax.block_until_ready(z)
    t0 = time.perf_counter()
    os_ = [f1(*dev_in, *z) for z in zs]
    jax.block_until_ready(os_)
    return (time.perf_counter() - t0) / n_iters * 1e9


def kernel(x1, src1, dst1, vals1, x2, src2, dst2, vals2,
           W1_0, b1_0, W1_1, b1_1, W2_0, b2_0, W2_1, b2_1):
    graphs = []
    for (x, src, dst, vals, W0, b0, W1, b1) in (
        (x1, src1, dst1, vals1, W1_0, b1_0, W1_1, b1_1),
        (x2, src2, dst2, vals2, W2_0, b2_0, W2_1, b2_1),
    ):
        streams, plan = _preprocess_graph(src, dst, vals)
        graphs.append((np.asarray(x, np.float32), streams, plan,
                       np.asarray(W0, np.float32), np.asarray(b0, np.float32),
                       np.asarray(W1, np.float32), np.asarray(b1, np.float32)))

    results = _build_and_run(graphs)

    out = np.zeros((2, N_NODES, F_OUT), np.float32)
    for g in (1, 2):
        for c in range(C):
            blk = results[c]["out"][:, (g - 1) * F_OUT:g * F_OUT]
            out[g - 1, c * SLAB:(c + 1) * SLAB, :] = np.asarray(blk, np.float32)
    return out
